# revision 21
# baseline (speedup 1.0000x reference)
"""Multi-head attention kernel for Trainium2, SPMD across 8 NeuronCores.

Problem: b=2, s=2048, d_model=1024, 16 heads x 64 dims, packed QKV proj,
softmax over keys (boolean key mask), out-projection.

Sharding: core c in 0..7 handles batch b = c//4 and a group of 4 heads
g = c%4 (data parallel over batch x tensor parallel over heads).  Each
core computes its head-group's out-projection partial [2048, 1024]; the
host sums the 4 partials per batch (row-parallel reduction done on host).

Device-side dataflow per core (all fp32):
  - QKV proj: weights stationary.  Q,K produced transposed [d, s] with two
    heads packed per SBUF tile ([128, 2048]: head A rows 0-63, head B rows
    64-127).  V produced in natural layout [s, d] as 16 tiles [128, 4*65]
    with a ones-column appended per head (col 64) for the softmax rowsum;
    masked key rows of V (and the ones col) are zeroed with a per-partition
    scalar multiply, which implements -inf score masking exactly.
  - Scores transposed St[sk, sq] = K @ Q^T per 128-key tile; the two heads
    of a pair run as row-tiled concurrent matmuls (tile_position rows 0/64)
    writing the two halves of one [128, 2048] PSUM tile.
  - exp on ScalarE (scale=1/8 folded in), one [128, 2048] instr per k-tile.
  - PV: out^T[65, sq] accumulated over k-tiles in PSUM; row 64 = rowsum.
  - normalize: DVE reciprocal of rowsum, GPSIMD partition_broadcast to 64
    rows, DVE multiply -> O^T tiles packed per pair ([128, 2048]).
  - out-proj: stationary = packed O^T s-slices, moving = W_out^T slices,
    both pairs accumulated in PSUM; evict via DVE; DMA to DRAM.
"""

import numpy as np
import ml_dtypes

BF = ml_dtypes.bfloat16
S = 2048
C = 1024
DQ = 64
HL = 4  # local heads per core
KT = S // 128  # 16 key tiles
CT = C // 128  # 8 contraction tiles
SCALE = 8.0  # sqrt(DQ)

_CACHED = None


def _build():
    import concourse.bacc as bacc
    import concourse.mybir as mybir
    import concourse.tile as tile
    from concourse.tile_rust import add_dep_helper

    F32 = mybir.dt.float32
    BF16 = mybir.dt.bfloat16
    EXP = mybir.ActivationFunctionType.Exp

    nc = bacc.Bacc(
        "TRN2",
        target_bir_lowering=False,
        debug=False,
        enable_asserts=False,
        num_devices=8,
    )

    XT = nc.dram_tensor("xt", [C, S], BF16, kind="ExternalInput").ap()
    WQ = nc.dram_tensor("wq", [C, 2 * 128], BF16, kind="ExternalInput").ap()
    WK = nc.dram_tensor("wk", [C, 2 * 128], BF16, kind="ExternalInput").ap()
    WV = nc.dram_tensor("wv", [C, 2 * 128], BF16, kind="ExternalInput").ap()
    WO = nc.dram_tensor("wo", [HL * DQ, C], BF16, kind="ExternalInput").ap()
    MV = nc.dram_tensor("maskv", [128, KT], F32, kind="ExternalInput").ap()
    OUT = nc.dram_tensor("out", [S, C], F32, kind="ExternalOutput").ap()

    with tile.TileContext(nc) as tc:
        with (
            tc.tile_pool(name="xt", bufs=CT) as p_xt,
            tc.tile_pool(name="wqk", bufs=2) as p_w,
            tc.tile_pool(name="wv", bufs=CT) as p_wv,
            tc.tile_pool(name="wo", bufs=2) as p_wo,
            tc.tile_pool(name="cst", bufs=1) as p_c,
            tc.tile_pool(name="qk", bufs=4) as p_qk,
            tc.tile_pool(name="v", bufs=KT) as p_v,
            tc.tile_pool(name="pt", bufs=18) as p_pt,
            tc.tile_pool(name="r", bufs=1) as p_r,
            tc.tile_pool(name="bc", bufs=1) as p_bc,
            tc.tile_pool(name="ot", bufs=2) as p_ot,
            tc.tile_pool(name="sc", bufs=1) as p_sc,
            tc.tile_pool(name="os", bufs=4) as p_os,
            tc.tile_pool(name="psA", bufs=2, space="PSUM") as psA,
            tc.tile_pool(name="psB", bufs=2, space="PSUM") as psB,
        ):
            # ---------------- input DMA ----------------
            xt_t = []
            for c in range(CT):
                t = p_xt.tile([128, S], BF16, tag="xt", name="xt_t")
                nc.sync.dma_start(t[:], XT[c * 128 : (c + 1) * 128, :])
                xt_t.append(t)
            wv_t = []
            for c in range(CT):
                t = p_wv.tile([128, HL * DQ], BF16, tag="wv", name="wv_t")
                nc.sync.dma_start(t[:], WV[c * 128 : (c + 1) * 128, :])
                wv_t.append(t)
            wo_t = []
            for p in range(2):
                t = p_wo.tile([128, C], BF16, tag="wo", name="wo_t")
                nc.sync.dma_start(t[:], WO[p * 128 : (p + 1) * 128, :])
                wo_t.append(t)
            mv_t = p_c.tile([128, KT], F32, tag="mv", name="mv_t")
            nc.sync.dma_start(mv_t[:], MV[:])
            # Q/K weights prestaged whole: dst[p, c*256+j] = w[c*128+p, j]
            wq_sb = p_w.tile([128, CT * 256], BF16, tag="wq", name="wq_sb")
            wk_sb = p_w.tile([128, CT * 256], BF16, tag="wk", name="wk_sb")
            for wsb, wsrc in ((wq_sb, WQ), (wk_sb, WK)):
                nc.sync.dma_start(
                    wsb[:, 0 : CT * 256].rearrange("p (c j) -> p c j", j=256),
                    wsrc.rearrange("(c p) j -> p c j", p=128),
                )

            # ---------------- QKV projection ----------------
            # Q,K transposed layout: per pair a [128, 2048] tile
            # (rows 0-63 head 2p, rows 64-127 head 2p+1).
            # Order: pair-0 Q,K then V (unblocks attention pair 0), then
            # pair-1 Q,K (overlaps attention pair 0 on the PE).
            qk_tiles = {}

            def proj_qk(nm, wsb, pair, pools):
                dst = p_qk.tile([128, S], BF16, tag="qk", name="qk_t")
                qk_tiles[(nm, pair)] = dst
                ps_tiles = [
                    (pools[0].tile([128, 1024], F32, tag=pools[1], name="pp"), 0),
                    (pools[0].tile([128, 1024], F32, tag=pools[1], name="pp"), 1024),
                ]
                for c in range(CT):
                    wt = wsb[:, c * 256 + pair * 128 : c * 256 + (pair + 1) * 128]
                    for pst, off in ps_tiles:
                        for n in range(2):
                            nc.tensor.matmul(
                                pst[:, n * 512 : (n + 1) * 512],
                                lhsT=wt,
                                rhs=xt_t[c][:, off + n * 512 : off + (n + 1) * 512],
                                start=(c == 0),
                                stop=(c == CT - 1),
                            )
                for pst, off in ps_tiles:
                    nc.vector.tensor_copy(dst[:, off : off + 1024], pst[:, 0:1024])

            def proj_v():
                for st in range(KT):
                    psv = psB.tile([128, HL * DQ], F32, tag="B", name="psv")
                    for c in range(CT):
                        nc.tensor.matmul(
                            psv[:, 0 : HL * DQ],
                            lhsT=xt_t[c][:, st * 128 : (st + 1) * 128],
                            rhs=wv_t[c][:],
                            start=(c == 0),
                            stop=(c == CT - 1),
                        )
                    vt = p_v.tile([128, HL * 65], BF16, tag="v", name="v_t")
                    v3 = vt[:, 0 : HL * 65].rearrange("p (h c) -> p h c", c=65)
                    s3 = psv[:, 0 : HL * DQ].rearrange("p (h c) -> p h c", c=DQ)
                    nc.vector.tensor_copy(v3[:, :, 0:DQ], s3[:, :, :])
                    nc.vector.memset(v3[:, :, DQ : DQ + 1], 1.0)
                    nc.vector.tensor_scalar_mul(vt[:], vt[:], mv_t[:, st : st + 1])
                    v_t.append(vt)

            v_t = []
            proj_qk("q", wq_sb, 0, (psA, "A"))
            proj_qk("k", wk_sb, 0, (psB, "B"))
            proj_qk("q", wq_sb, 1, (psA, "A"))
            proj_qk("k", wk_sb, 1, (psB, "B"))
            # V projection is emitted INSIDE the first attention j-loop
            # (see attention()), so the exp stream starts right after the
            # Q/K projections and V projects under the first 8 exps.

            # ---------------- attention ----------------
            # Per (pair, j-half): ping-pong St tiles [128,1024] per head so
            # exp (ScalarE) streams back-to-back while the PE computes the
            # next scores; PV trails PIPE iterations behind so a blocked acc
            # slot at a j-boundary doesn't head-of-line-block St in the PE
            # FIFO.
            PIPE = 3
            rt = p_r.tile([65, S], F32, tag="r", name="r_t")
            bct = p_bc.tile([64, 1024], F32, tag="bc", name="bc_t")
            bc2 = p_bc.tile([64, 1024], F32, tag="bc2", name="bc2_t")
            ot_tiles = []
            scr = p_sc.tile([64, S], BF16, tag="sc", name="sc_t")

            def attention(pair):
                qt = qk_tiles[("q", pair)]
                kt = qk_tiles[("k", pair)]
                ot = p_ot.tile([128, S], BF16, tag="ot", name="ot_t")
                ot_tiles.append(ot)
                hA, hB = 2 * pair, 2 * pair + 1
                for j in range(2):
                    # On the very first j-loop, inject the V and pair-1 Q/K
                    # projections into the PE stream after the first 8
                    # score tiles, so exp runs while they project.  PV then
                    # trails by 8 (it needs V).
                    inject = pair == 0 and j == 0
                    pipe = 8 if inject else PIPE
                    jo = j * 1024
                    accs = []
                    pts = {}

                    def st_exp(k):
                        for i, base in enumerate((0, 64)):
                            stp = psA.tile([128, 1024], F32, tag="A", name="stp")
                            for n in range(2):
                                nc.tensor.matmul(
                                    stp[:, n * 512 : (n + 1) * 512],
                                    lhsT=kt[base : base + DQ, k * 128 : (k + 1) * 128],
                                    rhs=qt[base : base + DQ, jo + n * 512 : jo + (n + 1) * 512],
                                    start=True,
                                    stop=True,
                                )
                            pt = p_pt.tile([128, 1024], BF16, tag="pt", name="pt_t")
                            nc.scalar.activation(pt[:], stp[:], EXP, scale=1.0 / SCALE)
                            pts[(k, i)] = pt

                    def pv(k):
                        if not accs:
                            accs.append(psB.tile([65, 1024], F32, tag="B", name="acc"))
                            accs.append(psB.tile([65, 1024], F32, tag="B", name="acc"))
                        for i, h in enumerate((hA, hB)):
                            pt = pts.pop((k, i))
                            for n in range(2):
                                nc.tensor.matmul(
                                    accs[i][0:65, n * 512 : (n + 1) * 512],
                                    lhsT=v_t[k][:, h * 65 : h * 65 + 65],
                                    rhs=pt[:, n * 512 : (n + 1) * 512],
                                    start=(k == 0),
                                    stop=(k == KT - 1),
                                )

                    for k in range(KT):
                        st_exp(k)
                        if inject and k == 7:
                            proj_v()
                        if k >= pipe:
                            pv(k - pipe)
                    for k in range(KT - pipe, KT):
                        pv(k)

                    # normalize:  O = PV / rowsum  (rowsum in acc row 64).
                    # partition_broadcast's ucode reads via gpsimd core 0,
                    # which only sees physical partitions 0-15 -> the source
                    # row must sit on partition 0; DMA-hop it there first.
                    for acc, dst in ((accs[0], ot[0:64, jo : jo + 1024]),
                                     (accs[1], scr[0:64, jo : jo + 1024])):
                        nc.vector.tensor_copy(
                            rt[64:65, jo : jo + 1024], acc[64:65, 0:1024]
                        )
                        nc.sync.dma_start(
                            rt[0:1, jo : jo + 1024], rt[64:65, jo : jo + 1024]
                        )
                        nc.gpsimd.partition_broadcast(
                            bct[0:64, 0:1024], rt[0:1, jo : jo + 1024]
                        )
                        nc.vector.reciprocal_approx_fast(
                            bc2[0:64, 0:1024], bct[0:64, 0:1024]
                        )
                        nc.vector.tensor_mul(dst, acc[0:64, 0:1024], bc2[0:64, 0:1024])
                    # pack head B into rows 64..127 of the pair's O tile
                    nc.sync.dma_start(
                        ot[64:128, jo : jo + 1024], scr[0:64, jo : jo + 1024]
                    )

            attention(0)
            attention(1)

            # ---------------- out-projection ----------------
            ps_cycle = [(psA, "A"), (psB, "B")]
            for st in range(KT):
                pool, tag = ps_cycle[st % 2]
                po = pool.tile([128, C], F32, tag=tag, name="po")
                for p in range(2):
                    for n in range(2):
                        nc.tensor.matmul(
                            po[:, n * 512 : (n + 1) * 512],
                            lhsT=ot_tiles[p][:, st * 128 : (st + 1) * 128],
                            rhs=wo_t[p][:, n * 512 : (n + 1) * 512],
                            start=(p == 0),
                            stop=(p == 1),
                        )
                os_t = p_os.tile([128, C], F32, tag="os", name="os_t")
                nc.vector.tensor_copy(os_t[:, 0:512], po[:, 0:512])
                nc.scalar.copy(os_t[:, 512:1024], po[:, 512:1024])
                nc.sync.dma_start(OUT[st * 128 : (st + 1) * 128, :], os_t[:])

    nc.compile()
    return nc


def _get_nc():
    global _CACHED
    if _CACHED is None:
        _CACHED = _build()
    return _CACHED


def _prep_in_maps(X, W_qkv, W_out, mask):
    X = np.asarray(X, dtype=np.float32)
    Wqkv = np.asarray(W_qkv, dtype=np.float32)
    Wo = np.asarray(W_out, dtype=np.float32)
    m = np.asarray(mask)
    W3 = Wqkv.reshape(16, DQ, 3, C)
    in_maps = []
    for core in range(8):
        b = core // 4
        g = core % 4
        hs = slice(4 * g, 4 * g + 4)
        wq = np.ascontiguousarray(W3[hs, :, 0, :].reshape(HL * DQ, C).T.astype(BF))
        wk = np.ascontiguousarray(W3[hs, :, 1, :].reshape(HL * DQ, C).T.astype(BF))
        wv = np.ascontiguousarray(W3[hs, :, 2, :].reshape(HL * DQ, C).T.astype(BF))
        wo = np.ascontiguousarray(Wo[:, 256 * g : 256 * (g + 1)].T.astype(BF))
        xt = np.ascontiguousarray(X[b].T.astype(BF))
        mv = np.ascontiguousarray(
            m[b].astype(np.float32).reshape(KT, 128).T
        )
        in_maps.append(
            {"xt": xt, "wq": wq, "wk": wk, "wv": wv, "wo": wo, "maskv": mv}
        )
    return in_maps


def _run(in_maps, trace=False, **kw):
    from concourse import bass_utils

    nc = _get_nc()
    return bass_utils.run_bass_kernel_spmd(
        nc, in_maps, core_ids=list(range(8)), trace=trace, **kw
    )


def _gather(results):
    out = np.empty((2, S, C), dtype=np.float32)
    out[0] = results[0]["out"] + results[1]["out"] + results[2]["out"] + results[3]["out"]
    out[1] = results[4]["out"] + results[5]["out"] + results[6]["out"] + results[7]["out"]
    return out


def kernel(X, W_qkv, W_out, mask):
    in_maps = _prep_in_maps(X, W_qkv, W_out, mask)
    res = _run(in_maps)
    return _gather(res.results)


# revision 23
# speedup vs baseline: 1.0047x; 1.0047x over previous
"""Multi-head attention kernel for Trainium2, SPMD across 8 NeuronCores.

Problem: b=2, s=2048, d_model=1024, 16 heads x 64 dims, packed QKV proj,
softmax over keys (boolean key mask), out-projection.

Sharding: core c in 0..7 handles batch b = c//4 and a group of 4 heads
g = c%4 (data parallel over batch x tensor parallel over heads).  Each
core computes its head-group's out-projection partial [2048, 1024]; the
host sums the 4 partials per batch (row-parallel reduction done on host).

Device-side dataflow per core (all fp32):
  - QKV proj: weights stationary.  Q,K produced transposed [d, s] with two
    heads packed per SBUF tile ([128, 2048]: head A rows 0-63, head B rows
    64-127).  V produced in natural layout [s, d] as 16 tiles [128, 4*65]
    with a ones-column appended per head (col 64) for the softmax rowsum;
    masked key rows of V (and the ones col) are zeroed with a per-partition
    scalar multiply, which implements -inf score masking exactly.
  - Scores transposed St[sk, sq] = K @ Q^T per 128-key tile; the two heads
    of a pair run as row-tiled concurrent matmuls (tile_position rows 0/64)
    writing the two halves of one [128, 2048] PSUM tile.
  - exp on ScalarE (scale=1/8 folded in), one [128, 2048] instr per k-tile.
  - PV: out^T[65, sq] accumulated over k-tiles in PSUM; row 64 = rowsum.
  - normalize: DVE reciprocal of rowsum, GPSIMD partition_broadcast to 64
    rows, DVE multiply -> O^T tiles packed per pair ([128, 2048]).
  - out-proj: stationary = packed O^T s-slices, moving = W_out^T slices,
    both pairs accumulated in PSUM; evict via DVE; DMA to DRAM.
"""

import numpy as np
import ml_dtypes

BF = ml_dtypes.bfloat16
S = 2048
C = 1024
DQ = 64
HL = 4  # local heads per core
KT = S // 128  # 16 key tiles
CT = C // 128  # 8 contraction tiles
SCALE = 8.0  # sqrt(DQ)

_CACHED = None


def _build():
    import concourse.bacc as bacc
    import concourse.mybir as mybir
    import concourse.tile as tile
    from concourse.tile_rust import add_dep_helper

    F32 = mybir.dt.float32
    BF16 = mybir.dt.bfloat16
    EXP = mybir.ActivationFunctionType.Exp

    nc = bacc.Bacc(
        "TRN2",
        target_bir_lowering=False,
        debug=False,
        enable_asserts=False,
        num_devices=8,
    )

    XT = nc.dram_tensor("xt", [C, S], BF16, kind="ExternalInput").ap()
    WQ = nc.dram_tensor("wq", [128, CT * 256], BF16, kind="ExternalInput").ap()
    WK = nc.dram_tensor("wk", [128, CT * 256], BF16, kind="ExternalInput").ap()
    WV = nc.dram_tensor("wv", [C, 2 * 128], BF16, kind="ExternalInput").ap()
    WO = nc.dram_tensor("wo", [HL * DQ, C], BF16, kind="ExternalInput").ap()
    MV = nc.dram_tensor("maskv", [128, KT], F32, kind="ExternalInput").ap()
    OUT = nc.dram_tensor("out", [S, C], F32, kind="ExternalOutput").ap()

    with tile.TileContext(nc) as tc:
        with (
            tc.tile_pool(name="xt", bufs=CT) as p_xt,
            tc.tile_pool(name="wqk", bufs=2) as p_w,
            tc.tile_pool(name="wv", bufs=CT) as p_wv,
            tc.tile_pool(name="wo", bufs=2) as p_wo,
            tc.tile_pool(name="cst", bufs=1) as p_c,
            tc.tile_pool(name="qk", bufs=4) as p_qk,
            tc.tile_pool(name="v", bufs=KT) as p_v,
            tc.tile_pool(name="pt", bufs=18) as p_pt,
            tc.tile_pool(name="r", bufs=1) as p_r,
            tc.tile_pool(name="bc", bufs=1) as p_bc,
            tc.tile_pool(name="ot", bufs=2) as p_ot,
            tc.tile_pool(name="sc", bufs=1) as p_sc,
            tc.tile_pool(name="os", bufs=4) as p_os,
            tc.tile_pool(name="psA", bufs=2, space="PSUM") as psA,
            tc.tile_pool(name="psB", bufs=2, space="PSUM") as psB,
        ):
            # ---------------- input DMA ----------------
            xt_t = []
            for c in range(CT):
                t = p_xt.tile([128, S], BF16, tag="xt", name="xt_t")
                nc.sync.dma_start(t[:], XT[c * 128 : (c + 1) * 128, :])
                xt_t.append(t)
            wv_t = []
            for c in range(CT):
                t = p_wv.tile([128, HL * DQ], BF16, tag="wv", name="wv_t")
                nc.sync.dma_start(t[:], WV[c * 128 : (c + 1) * 128, :])
                wv_t.append(t)
            wo_t = []
            for p in range(2):
                t = p_wo.tile([128, C], BF16, tag="wo", name="wo_t")
                nc.sync.dma_start(t[:], WO[p * 128 : (p + 1) * 128, :])
                wo_t.append(t)
            mv_t = p_c.tile([128, KT], F32, tag="mv", name="mv_t")
            nc.sync.dma_start(mv_t[:], MV[:])
            # Q/K weights prestaged whole: dst[p, c*256+j] = w[c*128+p, j]
            wq_sb = p_w.tile([128, CT * 256], BF16, tag="wq", name="wq_sb")
            wk_sb = p_w.tile([128, CT * 256], BF16, tag="wk", name="wk_sb")
            for wsb, wsrc in ((wq_sb, WQ), (wk_sb, WK)):
                nc.sync.dma_start(wsb[:], wsrc[:])

            # ---------------- QKV projection ----------------
            # Q,K transposed layout: per pair a [128, 2048] tile
            # (rows 0-63 head 2p, rows 64-127 head 2p+1).
            # Order: pair-0 Q,K then V (unblocks attention pair 0), then
            # pair-1 Q,K (overlaps attention pair 0 on the PE).
            qk_tiles = {}

            def proj_qk(nm, wsb, pair, pools):
                dst = p_qk.tile([128, S], BF16, tag="qk", name="qk_t")
                qk_tiles[(nm, pair)] = dst
                ps_tiles = [
                    (pools[0].tile([128, 1024], F32, tag=pools[1], name="pp"), 0),
                    (pools[0].tile([128, 1024], F32, tag=pools[1], name="pp"), 1024),
                ]
                for c in range(CT):
                    wt = wsb[:, c * 256 + pair * 128 : c * 256 + (pair + 1) * 128]
                    for pst, off in ps_tiles:
                        for n in range(2):
                            nc.tensor.matmul(
                                pst[:, n * 512 : (n + 1) * 512],
                                lhsT=wt,
                                rhs=xt_t[c][:, off + n * 512 : off + (n + 1) * 512],
                                start=(c == 0),
                                stop=(c == CT - 1),
                            )
                for pst, off in ps_tiles:
                    nc.vector.tensor_copy(dst[:, off : off + 1024], pst[:, 0:1024])

            def proj_v():
                for st in range(KT):
                    psv = psB.tile([128, HL * DQ], F32, tag="B", name="psv")
                    for c in range(CT):
                        nc.tensor.matmul(
                            psv[:, 0 : HL * DQ],
                            lhsT=xt_t[c][:, st * 128 : (st + 1) * 128],
                            rhs=wv_t[c][:],
                            start=(c == 0),
                            stop=(c == CT - 1),
                        )
                    vt = p_v.tile([128, HL * 65], BF16, tag="v", name="v_t")
                    v3 = vt[:, 0 : HL * 65].rearrange("p (h c) -> p h c", c=65)
                    s3 = psv[:, 0 : HL * DQ].rearrange("p (h c) -> p h c", c=DQ)
                    nc.vector.tensor_copy(v3[:, :, 0:DQ], s3[:, :, :])
                    nc.vector.memset(v3[:, :, DQ : DQ + 1], 1.0)
                    nc.vector.tensor_scalar_mul(vt[:], vt[:], mv_t[:, st : st + 1])
                    v_t.append(vt)

            v_t = []
            proj_qk("q", wq_sb, 0, (psA, "A"))
            proj_qk("k", wk_sb, 0, (psB, "B"))
            proj_qk("q", wq_sb, 1, (psA, "A"))
            proj_qk("k", wk_sb, 1, (psB, "B"))
            # V projection is emitted INSIDE the first attention j-loop
            # (see attention()), so the exp stream starts right after the
            # Q/K projections and V projects under the first 8 exps.

            # ---------------- attention ----------------
            # Per (pair, j-half): ping-pong St tiles [128,1024] per head so
            # exp (ScalarE) streams back-to-back while the PE computes the
            # next scores; PV trails PIPE iterations behind so a blocked acc
            # slot at a j-boundary doesn't head-of-line-block St in the PE
            # FIFO.
            PIPE = 3
            rth = [p_r.tile([65, S], F32, tag="rA", name="r_t"),
                   p_r.tile([65, S], F32, tag="rB", name="r_t")]
            bct_i = [p_bc.tile([64, 1024], F32, tag="bcA", name="bc_t"),
                     p_bc.tile([64, 1024], F32, tag="bcB", name="bc_t")]
            bc2_i = [p_bc.tile([64, 1024], F32, tag="bc2A", name="bc2_t"),
                     p_bc.tile([64, 1024], F32, tag="bc2B", name="bc2_t")]
            ot_tiles = []
            scr = p_sc.tile([64, S], BF16, tag="sc", name="sc_t")

            def attention(pair):
                qt = qk_tiles[("q", pair)]
                kt = qk_tiles[("k", pair)]
                ot = p_ot.tile([128, S], BF16, tag="ot", name="ot_t")
                ot_tiles.append(ot)
                hA, hB = 2 * pair, 2 * pair + 1
                for j in range(2):
                    # On the very first j-loop, inject the V and pair-1 Q/K
                    # projections into the PE stream after the first 8
                    # score tiles, so exp runs while they project.  PV then
                    # trails by 8 (it needs V).
                    inject = pair == 0 and j == 0
                    pipe = 8 if inject else PIPE
                    jo = j * 1024
                    accs = []
                    pts = {}

                    def st_exp(k):
                        for i, base in enumerate((0, 64)):
                            stp = psA.tile([128, 1024], F32, tag="A", name="stp")
                            for n in range(2):
                                nc.tensor.matmul(
                                    stp[:, n * 512 : (n + 1) * 512],
                                    lhsT=kt[base : base + DQ, k * 128 : (k + 1) * 128],
                                    rhs=qt[base : base + DQ, jo + n * 512 : jo + (n + 1) * 512],
                                    start=True,
                                    stop=True,
                                )
                            pt = p_pt.tile([128, 1024], BF16, tag="pt", name="pt_t")
                            nc.scalar.activation(pt[:], stp[:], EXP, scale=1.0 / SCALE)
                            pts[(k, i)] = pt

                    def pv(k):
                        if not accs:
                            accs.append(psB.tile([65, 1024], F32, tag="B", name="acc"))
                            accs.append(psB.tile([65, 1024], F32, tag="B", name="acc"))
                        for i, h in enumerate((hA, hB)):
                            pt = pts.pop((k, i))
                            for n in range(2):
                                nc.tensor.matmul(
                                    accs[i][0:65, n * 512 : (n + 1) * 512],
                                    lhsT=v_t[k][:, h * 65 : h * 65 + 65],
                                    rhs=pt[:, n * 512 : (n + 1) * 512],
                                    start=(k == 0),
                                    stop=(k == KT - 1),
                                )

                    for k in range(KT):
                        st_exp(k)
                        if inject and k == 7:
                            proj_v()
                        if k >= pipe:
                            pv(k - pipe)
                    for k in range(KT - pipe, KT):
                        pv(k)

                    # normalize:  O = PV / rowsum  (rowsum in acc row 64).
                    # partition_broadcast's ucode reads via gpsimd core 0,
                    # which only sees physical partitions 0-15 -> the source
                    # row must sit on partition 0; DMA-hop it there first.
                    dsts = (ot[0:64, jo : jo + 1024], scr[0:64, jo : jo + 1024])
                    for i in range(2):
                        acc, dst = accs[i], dsts[i]
                        bct, bc2 = bct_i[i], bc2_i[i]
                        nc.vector.tensor_copy(
                            rth[i][64:65, jo : jo + 1024], acc[64:65, 0:1024]
                        )
                        nc.sync.dma_start(
                            rth[i][0:1, jo : jo + 1024], rth[i][64:65, jo : jo + 1024]
                        )
                        nc.gpsimd.partition_broadcast(
                            bct[0:64, 0:1024], rth[i][0:1, jo : jo + 1024]
                        )
                        nc.vector.reciprocal_approx_fast(
                            bc2[0:64, 0:1024], bct[0:64, 0:1024]
                        )
                        nc.vector.tensor_mul(dst, acc[0:64, 0:1024], bc2[0:64, 0:1024])
                    # pack head B into rows 64..127 of the pair's O tile
                    nc.sync.dma_start(
                        ot[64:128, jo : jo + 1024], scr[0:64, jo : jo + 1024]
                    )

            attention(0)
            attention(1)

            # ---------------- out-projection ----------------
            ps_cycle = [(psA, "A"), (psB, "B")]
            for st in range(KT):
                pool, tag = ps_cycle[st % 2]
                po = pool.tile([128, C], F32, tag=tag, name="po")
                for p in range(2):
                    for n in range(2):
                        nc.tensor.matmul(
                            po[:, n * 512 : (n + 1) * 512],
                            lhsT=ot_tiles[p][:, st * 128 : (st + 1) * 128],
                            rhs=wo_t[p][:, n * 512 : (n + 1) * 512],
                            start=(p == 0),
                            stop=(p == 1),
                        )
                os_t = p_os.tile([128, C], F32, tag="os", name="os_t")
                nc.vector.tensor_copy(os_t[:, 0:512], po[:, 0:512])
                nc.scalar.copy(os_t[:, 512:1024], po[:, 512:1024])
                nc.sync.dma_start(OUT[st * 128 : (st + 1) * 128, :], os_t[:])

    nc.compile()
    return nc


def _get_nc():
    global _CACHED
    if _CACHED is None:
        _CACHED = _build()
    return _CACHED


def _prep_in_maps(X, W_qkv, W_out, mask):
    X = np.asarray(X, dtype=np.float32)
    Wqkv = np.asarray(W_qkv, dtype=np.float32)
    Wo = np.asarray(W_out, dtype=np.float32)
    m = np.asarray(mask)
    W3 = Wqkv.reshape(16, DQ, 3, C)
    in_maps = []
    for core in range(8):
        b = core // 4
        g = core % 4
        hs = slice(4 * g, 4 * g + 4)
        wq = W3[hs, :, 0, :].reshape(HL * DQ, C).T.astype(BF)
        wk = W3[hs, :, 1, :].reshape(HL * DQ, C).T.astype(BF)
        # pre-arrange for contiguous SBUF prestage: [128, c*256+j]
        wq = np.ascontiguousarray(
            wq.reshape(CT, 128, HL * DQ).transpose(1, 0, 2).reshape(128, CT * 256)
        )
        wk = np.ascontiguousarray(
            wk.reshape(CT, 128, HL * DQ).transpose(1, 0, 2).reshape(128, CT * 256)
        )
        wv = np.ascontiguousarray(W3[hs, :, 2, :].reshape(HL * DQ, C).T.astype(BF))
        wo = np.ascontiguousarray(Wo[:, 256 * g : 256 * (g + 1)].T.astype(BF))
        xt = np.ascontiguousarray(X[b].T.astype(BF))
        mv = np.ascontiguousarray(
            m[b].astype(np.float32).reshape(KT, 128).T
        )
        in_maps.append(
            {"xt": xt, "wq": wq, "wk": wk, "wv": wv, "wo": wo, "maskv": mv}
        )
    return in_maps


def _run(in_maps, trace=False, **kw):
    from concourse import bass_utils

    nc = _get_nc()
    return bass_utils.run_bass_kernel_spmd(
        nc, in_maps, core_ids=list(range(8)), trace=trace, **kw
    )


def _gather(results):
    out = np.empty((2, S, C), dtype=np.float32)
    out[0] = results[0]["out"] + results[1]["out"] + results[2]["out"] + results[3]["out"]
    out[1] = results[4]["out"] + results[5]["out"] + results[6]["out"] + results[7]["out"]
    return out


def kernel(X, W_qkv, W_out, mask):
    in_maps = _prep_in_maps(X, W_qkv, W_out, mask)
    res = _run(in_maps)
    return _gather(res.results)


# revision 24
# speedup vs baseline: 1.0681x; 1.0631x over previous
"""Multi-head attention kernel for Trainium2, SPMD across 8 NeuronCores.

Problem: b=2, s=2048, d_model=1024, 16 heads x 64 dims, packed QKV proj,
softmax over keys (boolean key mask), out-projection.

Sharding: core c in 0..7 handles batch b = c//4 and a group of 4 heads
g = c%4 (data parallel over batch x tensor parallel over heads).  Each
core computes its head-group's out-projection partial [2048, 1024]; the
host sums the 4 partials per batch (row-parallel reduction done on host).

Device-side dataflow per core (all fp32):
  - QKV proj: weights stationary.  Q,K produced transposed [d, s] with two
    heads packed per SBUF tile ([128, 2048]: head A rows 0-63, head B rows
    64-127).  V produced in natural layout [s, d] as 16 tiles [128, 4*65]
    with a ones-column appended per head (col 64) for the softmax rowsum;
    masked key rows of V (and the ones col) are zeroed with a per-partition
    scalar multiply, which implements -inf score masking exactly.
  - Scores transposed St[sk, sq] = K @ Q^T per 128-key tile; the two heads
    of a pair run as row-tiled concurrent matmuls (tile_position rows 0/64)
    writing the two halves of one [128, 2048] PSUM tile.
  - exp on ScalarE (scale=1/8 folded in), one [128, 2048] instr per k-tile.
  - PV: out^T[65, sq] accumulated over k-tiles in PSUM; row 64 = rowsum.
  - normalize: DVE reciprocal of rowsum, GPSIMD partition_broadcast to 64
    rows, DVE multiply -> O^T tiles packed per pair ([128, 2048]).
  - out-proj: stationary = packed O^T s-slices, moving = W_out^T slices,
    both pairs accumulated in PSUM; evict via DVE; DMA to DRAM.
"""

import numpy as np
import ml_dtypes

BF = ml_dtypes.bfloat16
S = 2048
C = 1024
DQ = 64
HL = 4  # local heads per core
KT = S // 128  # 16 key tiles
CT = C // 128  # 8 contraction tiles
SCALE = 8.0  # sqrt(DQ)

_CACHED = None


def _build():
    import concourse.bacc as bacc
    import concourse.mybir as mybir
    import concourse.tile as tile
    from concourse.tile_rust import add_dep_helper

    F32 = mybir.dt.float32
    BF16 = mybir.dt.bfloat16
    EXP = mybir.ActivationFunctionType.Exp

    nc = bacc.Bacc(
        "TRN2",
        target_bir_lowering=False,
        debug=False,
        enable_asserts=False,
        num_devices=8,
    )

    XT = nc.dram_tensor("xt", [C, S], BF16, kind="ExternalInput").ap()
    WQ = nc.dram_tensor("wq", [128, CT * 256], BF16, kind="ExternalInput").ap()
    WK = nc.dram_tensor("wk", [128, CT * 256], BF16, kind="ExternalInput").ap()
    WV = nc.dram_tensor("wv", [C, 2 * 128], BF16, kind="ExternalInput").ap()
    WO = nc.dram_tensor("wo", [HL * DQ, C], BF16, kind="ExternalInput").ap()
    MV = nc.dram_tensor("maskv", [128, KT], F32, kind="ExternalInput").ap()
    OUT = nc.dram_tensor("out", [S, C], F32, kind="ExternalOutput").ap()

    with tile.TileContext(nc) as tc:
        with (
            tc.tile_pool(name="xt", bufs=CT) as p_xt,
            tc.tile_pool(name="wqk", bufs=2) as p_w,
            tc.tile_pool(name="wv", bufs=CT) as p_wv,
            tc.tile_pool(name="wo", bufs=2) as p_wo,
            tc.tile_pool(name="cst", bufs=1) as p_c,
            tc.tile_pool(name="qk", bufs=4) as p_qk,
            tc.tile_pool(name="v", bufs=KT) as p_v,
            tc.tile_pool(name="pt", bufs=18) as p_pt,
            tc.tile_pool(name="r", bufs=1) as p_r,
            tc.tile_pool(name="bc", bufs=1) as p_bc,
            tc.tile_pool(name="ot", bufs=2) as p_ot,
            tc.tile_pool(name="sc", bufs=1) as p_sc,
            tc.tile_pool(name="os", bufs=4) as p_os,
            tc.tile_pool(name="psA", bufs=2, space="PSUM") as psA,
            tc.tile_pool(name="psB", bufs=2, space="PSUM") as psB,
        ):
            # ---------------- input DMA ----------------
            # Order matters: the first projection matmul waits on wq + xt[0],
            # so emit those DMAs first; wv/wo/mask are needed much later.
            wq_sb = p_w.tile([128, CT * 256], BF16, tag="wq", name="wq_sb")
            wk_sb = p_w.tile([128, CT * 256], BF16, tag="wk", name="wk_sb")
            for wsb, wsrc in ((wq_sb, WQ), (wk_sb, WK)):
                nc.sync.dma_start(wsb[:], wsrc[:])
            xt_t = []
            for c in range(CT):
                t = p_xt.tile([128, S], BF16, tag="xt", name="xt_t")
                nc.sync.dma_start(t[:], XT[c * 128 : (c + 1) * 128, :])
                xt_t.append(t)
            wv_t = []
            for c in range(CT):
                t = p_wv.tile([128, HL * DQ], BF16, tag="wv", name="wv_t")
                nc.sync.dma_start(t[:], WV[c * 128 : (c + 1) * 128, :])
                wv_t.append(t)
            mv_t = p_c.tile([128, KT], F32, tag="mv", name="mv_t")
            nc.sync.dma_start(mv_t[:], MV[:])
            wo_t = []
            for p in range(2):
                t = p_wo.tile([128, C], BF16, tag="wo", name="wo_t")
                nc.sync.dma_start(t[:], WO[p * 128 : (p + 1) * 128, :])
                wo_t.append(t)

            # ---------------- QKV projection ----------------
            # Q,K transposed layout: per pair a [128, 2048] tile
            # (rows 0-63 head 2p, rows 64-127 head 2p+1).
            # Order: pair-0 Q,K then V (unblocks attention pair 0), then
            # pair-1 Q,K (overlaps attention pair 0 on the PE).
            qk_tiles = {}

            def proj_qk(nm, wsb, pair, pools):
                dst = p_qk.tile([128, S], BF16, tag="qk", name="qk_t")
                qk_tiles[(nm, pair)] = dst
                ps_tiles = [
                    (pools[0].tile([128, 1024], F32, tag=pools[1], name="pp"), 0),
                    (pools[0].tile([128, 1024], F32, tag=pools[1], name="pp"), 1024),
                ]
                for c in range(CT):
                    wt = wsb[:, c * 256 + pair * 128 : c * 256 + (pair + 1) * 128]
                    for pst, off in ps_tiles:
                        for n in range(2):
                            nc.tensor.matmul(
                                pst[:, n * 512 : (n + 1) * 512],
                                lhsT=wt,
                                rhs=xt_t[c][:, off + n * 512 : off + (n + 1) * 512],
                                start=(c == 0),
                                stop=(c == CT - 1),
                            )
                for pst, off in ps_tiles:
                    nc.vector.tensor_copy(dst[:, off : off + 1024], pst[:, 0:1024])

            def proj_v():
                for st in range(KT):
                    psv = psB.tile([128, HL * DQ], F32, tag="B", name="psv")
                    for c in range(CT):
                        nc.tensor.matmul(
                            psv[:, 0 : HL * DQ],
                            lhsT=xt_t[c][:, st * 128 : (st + 1) * 128],
                            rhs=wv_t[c][:],
                            start=(c == 0),
                            stop=(c == CT - 1),
                        )
                    vt = p_v.tile([128, HL * 65], BF16, tag="v", name="v_t")
                    v3 = vt[:, 0 : HL * 65].rearrange("p (h c) -> p h c", c=65)
                    s3 = psv[:, 0 : HL * DQ].rearrange("p (h c) -> p h c", c=DQ)
                    nc.vector.tensor_copy(v3[:, :, 0:DQ], s3[:, :, :])
                    nc.vector.memset(v3[:, :, DQ : DQ + 1], 1.0)
                    nc.vector.tensor_scalar_mul(vt[:], vt[:], mv_t[:, st : st + 1])
                    v_t.append(vt)

            v_t = []
            proj_qk("q", wq_sb, 0, (psA, "A"))
            proj_qk("k", wk_sb, 0, (psB, "B"))
            proj_qk("q", wq_sb, 1, (psA, "A"))
            proj_qk("k", wk_sb, 1, (psB, "B"))
            # V projection is emitted INSIDE the first attention j-loop
            # (see attention()), so the exp stream starts right after the
            # Q/K projections and V projects under the first 8 exps.

            # ---------------- attention ----------------
            # Per (pair, j-half): ping-pong St tiles [128,1024] per head so
            # exp (ScalarE) streams back-to-back while the PE computes the
            # next scores; PV trails PIPE iterations behind so a blocked acc
            # slot at a j-boundary doesn't head-of-line-block St in the PE
            # FIFO.
            PIPE = 3
            rth = [p_r.tile([65, S], F32, tag="rA", name="r_t"),
                   p_r.tile([65, S], F32, tag="rB", name="r_t")]
            bct_i = [p_bc.tile([64, 1024], F32, tag="bcA", name="bc_t"),
                     p_bc.tile([64, 1024], F32, tag="bcB", name="bc_t")]
            bc2_i = [p_bc.tile([64, 1024], F32, tag="bc2A", name="bc2_t"),
                     p_bc.tile([64, 1024], F32, tag="bc2B", name="bc2_t")]
            ot_tiles = []
            scr = p_sc.tile([64, S], BF16, tag="sc", name="sc_t")

            def attention(pair):
                qt = qk_tiles[("q", pair)]
                kt = qk_tiles[("k", pair)]
                ot = p_ot.tile([128, S], BF16, tag="ot", name="ot_t")
                ot_tiles.append(ot)
                hA, hB = 2 * pair, 2 * pair + 1
                for j in range(2):
                    # On the very first j-loop, inject the V and pair-1 Q/K
                    # projections into the PE stream after the first 8
                    # score tiles, so exp runs while they project.  PV then
                    # trails by 8 (it needs V).
                    inject = pair == 0 and j == 0
                    pipe = 8 if inject else PIPE
                    jo = j * 1024
                    accs = []
                    pts = {}

                    def st_exp(k):
                        for i, base in enumerate((0, 64)):
                            stp = psA.tile([128, 1024], F32, tag="A", name="stp")
                            for n in range(2):
                                nc.tensor.matmul(
                                    stp[:, n * 512 : (n + 1) * 512],
                                    lhsT=kt[base : base + DQ, k * 128 : (k + 1) * 128],
                                    rhs=qt[base : base + DQ, jo + n * 512 : jo + (n + 1) * 512],
                                    start=True,
                                    stop=True,
                                )
                            pt = p_pt.tile([128, 1024], BF16, tag="pt", name="pt_t")
                            nc.scalar.activation(pt[:], stp[:], EXP, scale=1.0 / SCALE)
                            pts[(k, i)] = pt

                    def pv(k):
                        if not accs:
                            accs.append(psB.tile([65, 1024], F32, tag="B", name="acc"))
                            accs.append(psB.tile([65, 1024], F32, tag="B", name="acc"))
                        for i, h in enumerate((hA, hB)):
                            pt = pts.pop((k, i))
                            for n in range(2):
                                nc.tensor.matmul(
                                    accs[i][0:65, n * 512 : (n + 1) * 512],
                                    lhsT=v_t[k][:, h * 65 : h * 65 + 65],
                                    rhs=pt[:, n * 512 : (n + 1) * 512],
                                    start=(k == 0),
                                    stop=(k == KT - 1),
                                )

                    for k in range(KT):
                        st_exp(k)
                        if inject and k == 7:
                            proj_v()
                        if k >= pipe:
                            pv(k - pipe)
                    for k in range(KT - pipe, KT):
                        pv(k)

                    # normalize:  O = PV / rowsum  (rowsum in acc row 64).
                    # partition_broadcast's ucode reads via gpsimd core 0,
                    # which only sees physical partitions 0-15 -> the source
                    # row must sit on partition 0; DMA-hop it there first.
                    dsts = (ot[0:64, jo : jo + 1024], scr[0:64, jo : jo + 1024])
                    for i in range(2):
                        acc, dst = accs[i], dsts[i]
                        bct, bc2 = bct_i[i], bc2_i[i]
                        nc.vector.tensor_copy(
                            rth[i][64:65, jo : jo + 1024], acc[64:65, 0:1024]
                        )
                        nc.sync.dma_start(
                            rth[i][0:1, jo : jo + 1024], rth[i][64:65, jo : jo + 1024]
                        )
                        nc.gpsimd.partition_broadcast(
                            bct[0:64, 0:1024], rth[i][0:1, jo : jo + 1024]
                        )
                        nc.vector.reciprocal_approx_fast(
                            bc2[0:64, 0:1024], bct[0:64, 0:1024]
                        )
                        nc.vector.tensor_mul(dst, acc[0:64, 0:1024], bc2[0:64, 0:1024])
                    # pack head B into rows 64..127 of the pair's O tile
                    nc.sync.dma_start(
                        ot[64:128, jo : jo + 1024], scr[0:64, jo : jo + 1024]
                    )

            attention(0)
            attention(1)

            # ---------------- out-projection ----------------
            ps_cycle = [(psA, "A"), (psB, "B")]
            for st in range(KT):
                pool, tag = ps_cycle[st % 2]
                po = pool.tile([128, C], F32, tag=tag, name="po")
                for p in range(2):
                    for n in range(2):
                        nc.tensor.matmul(
                            po[:, n * 512 : (n + 1) * 512],
                            lhsT=ot_tiles[p][:, st * 128 : (st + 1) * 128],
                            rhs=wo_t[p][:, n * 512 : (n + 1) * 512],
                            start=(p == 0),
                            stop=(p == 1),
                        )
                os_t = p_os.tile([128, C], F32, tag="os", name="os_t")
                nc.vector.tensor_copy(os_t[:, 0:512], po[:, 0:512])
                nc.scalar.copy(os_t[:, 512:1024], po[:, 512:1024])
                nc.sync.dma_start(OUT[st * 128 : (st + 1) * 128, :], os_t[:])

    nc.compile()
    return nc


def _get_nc():
    global _CACHED
    if _CACHED is None:
        _CACHED = _build()
    return _CACHED


def _prep_in_maps(X, W_qkv, W_out, mask):
    X = np.asarray(X, dtype=np.float32)
    Wqkv = np.asarray(W_qkv, dtype=np.float32)
    Wo = np.asarray(W_out, dtype=np.float32)
    m = np.asarray(mask)
    W3 = Wqkv.reshape(16, DQ, 3, C)
    in_maps = []
    for core in range(8):
        b = core // 4
        g = core % 4
        hs = slice(4 * g, 4 * g + 4)
        wq = W3[hs, :, 0, :].reshape(HL * DQ, C).T.astype(BF)
        wk = W3[hs, :, 1, :].reshape(HL * DQ, C).T.astype(BF)
        # pre-arrange for contiguous SBUF prestage: [128, c*256+j]
        wq = np.ascontiguousarray(
            wq.reshape(CT, 128, HL * DQ).transpose(1, 0, 2).reshape(128, CT * 256)
        )
        wk = np.ascontiguousarray(
            wk.reshape(CT, 128, HL * DQ).transpose(1, 0, 2).reshape(128, CT * 256)
        )
        wv = np.ascontiguousarray(W3[hs, :, 2, :].reshape(HL * DQ, C).T.astype(BF))
        wo = np.ascontiguousarray(Wo[:, 256 * g : 256 * (g + 1)].T.astype(BF))
        xt = np.ascontiguousarray(X[b].T.astype(BF))
        mv = np.ascontiguousarray(
            m[b].astype(np.float32).reshape(KT, 128).T
        )
        in_maps.append(
            {"xt": xt, "wq": wq, "wk": wk, "wv": wv, "wo": wo, "maskv": mv}
        )
    return in_maps


def _run(in_maps, trace=False, **kw):
    from concourse import bass_utils

    nc = _get_nc()
    return bass_utils.run_bass_kernel_spmd(
        nc, in_maps, core_ids=list(range(8)), trace=trace, **kw
    )


def _gather(results):
    out = np.empty((2, S, C), dtype=np.float32)
    out[0] = results[0]["out"] + results[1]["out"] + results[2]["out"] + results[3]["out"]
    out[1] = results[4]["out"] + results[5]["out"] + results[6]["out"] + results[7]["out"]
    return out


def kernel(X, W_qkv, W_out, mask):
    in_maps = _prep_in_maps(X, W_qkv, W_out, mask)
    res = _run(in_maps)
    return _gather(res.results)
